# revision 1
# baseline (speedup 1.0000x reference)
"""LINK forward (gnn message passing SpMM) on 8 TRN2 NeuronCores.

out[r, :] = W_bias + sum_{e: row[e]==r} W_weight.T[col[e], :]

Strategy (1D row-wise SpMM partitioning):
  - Core k owns output rows [k*12500, (k+1)*12500).
  - Edges bucketed to cores by row; within a core, sorted by
    (col-chunk, 128-row output tile). Col space split into 4 chunks of
    25000 so gather indices fit int16.
  - W.T is converted to bf16 on host; device gathers per-edge rows
    (gpsimd.dma_gather, 256B descriptors over 16 DMA engines).
  - Segment-sum via TensorEngine: one-hot selection matrices (built with
    iota + is_equal on DVE) matmul'd against gathered rows, accumulated
    in PSUM, then added into a SBUF-resident accumulator (bias folded
    into the first chunk pass).
"""

import sys

sys.path.insert(0, "/opt/trn_rl_repo")

import numpy as np
import ml_dtypes

import concourse.bass as bass
import concourse.tile as tile
from concourse import bacc, mybir
from concourse.bass_utils import run_bass_kernel_spmd

P = 128
D = 128            # channels
N = 100000         # nodes
NCORE = 8
RPC = N // NCORE   # rows per core = 12500
NT = (RPC + P - 1) // P          # output tiles per core = 98
LAST_TILE_ROWS = RPC - (NT - 1) * P  # 84
NCH = 4
CHSZ = 25000       # col chunk size (int16-safe)
SLICE = 1024       # gather indices per dma_gather call (ring cap ~2048 descs)
SB = SLICE // P    # blocks per full slice = 64

LAST_EXEC_NS = None

_CACHE = {}


def _prepare(edge_index):
    """Bucket/sort/pad edges. Returns (nbt [NCH,NT] block counts,
    Lc [NCH] stream lengths, idx_arrs[core][ch] int16, m_arrs[core][ch] f32)."""
    row = np.asarray(edge_index[0], dtype=np.int64)
    col = np.asarray(edge_index[1], dtype=np.int64)
    E = row.shape[0]

    core = row // RPC
    lrow = row - core * RPC
    t = lrow >> 7
    m = lrow & 127
    ch = col // CHSZ
    lcol = col - ch * CHSZ

    gid = (core * NCH + ch) * NT + t
    order = np.argsort(gid, kind="stable")
    gid_s = gid[order]
    lcol_s = lcol[order]
    m_s = m[order]

    ngroups = NCORE * NCH * NT
    cnt = np.bincount(gid, minlength=ngroups).reshape(NCORE, NCH, NT)
    nbt = np.maximum(1, -(-cnt.max(axis=0) // P))      # [NCH, NT]
    seg_len = nbt * P
    Lc = seg_len.sum(axis=1)                            # [NCH]
    seg_start = np.zeros((NCH, NT), np.int64)
    seg_start[:, 1:] = np.cumsum(seg_len, axis=1)[:, :-1]

    starts_flat = np.zeros(ngroups, np.int64)
    flat_cnt = cnt.reshape(-1)
    starts_flat[1:] = np.cumsum(flat_cnt)[:-1]
    rank = np.arange(E, dtype=np.int64) - starts_flat[gid_s]

    core_s = gid_s // (NCH * NT)
    ch_s = (gid_s // NT) % NCH
    t_s = gid_s % NT
    dest = seg_start[ch_s, t_s] + rank

    idx_arrs = [[None] * NCH for _ in range(NCORE)]
    m_arrs = [[None] * NCH for _ in range(NCORE)]
    for c in range(NCORE):
        cm = core_s == c
        for k in range(NCH):
            mask = cm & (ch_s == k)
            ia = np.zeros(Lc[k], np.int16)
            ma = np.full(Lc[k], 200.0, np.float32)
            d = dest[mask]
            ia[d] = lcol_s[mask].astype(np.int16)
            ma[d] = m_s[mask].astype(np.float32)
            idx_arrs[c][k] = ia
            m_arrs[c][k] = ma
    return nbt, Lc, idx_arrs, m_arrs


def _build(nbt, Lc):
    nc = bacc.Bacc("TRN2", target_bir_lowering=False, num_swdge_queues=4)
    wt = nc.dram_tensor("wt", [N, D], mybir.dt.bfloat16, kind="ExternalInput")
    bias = nc.dram_tensor("bias", [P, D], mybir.dt.float32, kind="ExternalInput")
    idx_d = [
        nc.dram_tensor(f"idx{k}", [P, int(Lc[k]) // 16], mybir.dt.int16,
                       kind="ExternalInput")
        for k in range(NCH)
    ]
    m_d = [
        nc.dram_tensor(f"m{k}", [P, int(Lc[k]) // P], mybir.dt.bfloat16,
                       kind="ExternalInput")
        for k in range(NCH)
    ]
    out = nc.dram_tensor("out", [RPC, D], mybir.dt.float32, kind="ExternalOutput")

    with tile.TileContext(nc) as tc:
        with tc.tile_pool(name="const", bufs=1) as cpool, \
             tc.tile_pool(name="idx", bufs=2) as ipool, \
             tc.tile_pool(name="mval", bufs=2) as mpool, \
             tc.tile_pool(name="g", bufs=4) as gpool, \
             tc.tile_pool(name="s", bufs=4) as spool, \
             tc.tile_pool(name="psum", bufs=6, space="PSUM") as pspool:

            iota16 = cpool.tile([P, P], mybir.dt.int16)
            nc.gpsimd.iota(iota16[:], pattern=[[1, P]], base=0, channel_multiplier=0)
            iota_bf = cpool.tile([P, P], mybir.dt.bfloat16)
            nc.vector.tensor_copy(iota_bf[:], iota16[:])
            bias_t = cpool.tile([P, D], mybir.dt.float32)
            nc.sync.dma_start(bias_t[:], bias[:])
            acc = cpool.tile([P, NT * D], mybir.dt.float32)

            for k in range(NCH):
                lck = int(Lc[k])
                idx_t = ipool.tile([P, lck // 16], mybir.dt.int16, tag="idx")
                nc.sync.dma_start(idx_t[:], idx_d[k][:])
                m_t = mpool.tile([P, lck // P], mybir.dt.bfloat16, tag="mval")
                nc.sync.dma_start(m_t[:], m_d[k][:])

                g_tiles = {}
                s_tiles = {}
                gq = [0]

                def ensure(s, k=k, idx_t=idx_t, m_t=m_t, g_tiles=g_tiles,
                           s_tiles=s_tiles, lck=lck, gq=gq):
                    if s in g_tiles:
                        return
                    n = min(SLICE, lck - s * SLICE)
                    nb_s = n // P
                    g = gpool.tile([P, nb_s, D], mybir.dt.bfloat16, tag="g")
                    nc.gpsimd.dma_gather(
                        g[:],
                        wt[k * CHSZ:(k + 1) * CHSZ, :],
                        idx_t[:, s * (SLICE // 16): s * (SLICE // 16) + n // 16],
                        n, n, D, queue_num=gq[0] % 4,
                    )
                    gq[0] += 1
                    st = spool.tile([P, nb_s * P], mybir.dt.bfloat16, tag="s")
                    nc.vector.tensor_tensor(
                        out=st[:].rearrange("p (b m) -> p b m", m=P),
                        in0=m_t[:, s * SB: s * SB + nb_s].unsqueeze(2)
                            .broadcast_to([P, nb_s, P]),
                        in1=iota_bf[:].unsqueeze(1).broadcast_to([P, nb_s, P]),
                        op=mybir.AluOpType.is_equal,
                    )
                    g_tiles[s] = g
                    s_tiles[s] = st

                b = 0
                for t in range(NT):
                    nb = int(nbt[k][t])
                    ps = pspool.tile([P, D], mybir.dt.float32, space="PSUM")
                    for j in range(nb):
                        s = b // SB
                        b_loc = b % SB
                        ensure(s)
                        nc.tensor.matmul(
                            out=ps[:],
                            lhsT=s_tiles[s][:, b_loc * P:(b_loc + 1) * P],
                            rhs=g_tiles[s][:, b_loc, :],
                            start=(j == 0),
                            stop=(j == nb - 1),
                        )
                        b += 1
                    acc_sl = acc[:, t * D:(t + 1) * D]
                    if k == 0:
                        nc.vector.tensor_tensor(
                            out=acc_sl, in0=ps[:], in1=bias_t[:],
                            op=mybir.AluOpType.add,
                        )
                    else:
                        nc.vector.tensor_tensor(
                            out=acc_sl, in0=acc_sl, in1=ps[:],
                            op=mybir.AluOpType.add,
                        )

            # acc[p, t, :] holds out row t*128+p
            nc.sync.dma_start(
                out[: (NT - 1) * P, :].rearrange("(t p) d -> p t d", p=P),
                acc[:].rearrange("p (t d) -> p t d", d=D)[:, : NT - 1, :],
            )
            nc.sync.dma_start(
                out[(NT - 1) * P:, :],
                acc[:LAST_TILE_ROWS, (NT - 1) * D: NT * D],
            )
    nc.compile()
    return nc


def kernel(x=None, edge_index=None, W_weight=None, W_bias=None, _trace=False):
    global LAST_EXEC_NS
    edge_index = np.asarray(edge_index)
    W_weight = np.asarray(W_weight, dtype=np.float32)
    W_bias = np.asarray(W_bias, dtype=np.float32)

    key = (edge_index.tobytes()[:4096], edge_index.shape)
    cached = _CACHE.get(key)
    if cached is None:
        nbt, Lc, idx_arrs, m_arrs = _prepare(edge_index)
        nc = _build(nbt, Lc)
        in_maps = []
        wt_bf = np.ascontiguousarray(W_weight.T).astype(ml_dtypes.bfloat16)
        bias_b = np.tile(W_bias[None, :], (P, 1)).astype(np.float32)
        for c in range(NCORE):
            im = {"wt": wt_bf, "bias": bias_b}
            for k in range(NCH):
                ia = idx_arrs[c][k]
                im[f"idx{k}"] = np.ascontiguousarray(
                    np.tile(ia.reshape(-1, 16).T, (8, 1))).astype(np.int16)
                im[f"m{k}"] = np.ascontiguousarray(
                    m_arrs[c][k].reshape(-1, P).T).astype(ml_dtypes.bfloat16)
            in_maps.append(im)
        _CACHE[key] = (nc, in_maps)
    else:
        nc, in_maps = cached

    res = run_bass_kernel_spmd(nc, in_maps, core_ids=list(range(NCORE)),
                               trace=_trace)
    LAST_EXEC_NS = res.exec_time_ns
    outp = np.concatenate([res.results[c]["out"] for c in range(NCORE)], axis=0)
    return outp.astype(np.float32)



# revision 2
# speedup vs baseline: 1.0959x; 1.0959x over previous
"""LINK forward (gnn message passing SpMM) on 8 TRN2 NeuronCores — v3.

out[r, :] = W_bias + sum_{e: row[e]==r} W_weight.T[col[e], :]

Row-sharded (core k owns rows [k*12500, (k+1)*12500)). Edges bucketed by
(chunk, tile) with a block schedule shared across cores; per-edge rows of
W.T (bf16) gathered from HBM via SWDGE dma_gather (1024 idx/call, 4
queues); segment-sum via one-hot selection matmuls accumulated in PSUM per
128-row tile; PSUM + bias -> f32 tile -> DMA straight to the output.
"""

import sys

sys.path.insert(0, "/opt/trn_rl_repo")

import numpy as np
import ml_dtypes

import concourse.bass as bass
import concourse.tile as tile
from concourse import bacc, mybir
from concourse.bass_utils import run_bass_kernel_spmd

P = 128
D = 128
N = 100000
NCORE = 8
RPC = N // NCORE               # 12500
NT = (RPC + P - 1) // P        # 98
LAST_TILE_ROWS = RPC - (NT - 1) * P  # 84
NCH = 4
CHSZ = 25000
CALL = 1024                    # hard ISA cap for dma_gather num_idxs
NBC = CALL // P                # 8 blocks per call

LAST_EXEC_NS = None
_CACHE = {}


def _prepare(edge_index):
    """Bucket/sort/pad edges. Returns (nbt [NCH,NT] block counts,
    Lc [NCH] padded stream lengths, idx_arrs[core][ch] int16,
    m_arrs[core][ch] f32). Stream slot j -> partition j%128, block j//128."""
    row = np.asarray(edge_index[0], dtype=np.int64)
    col = np.asarray(edge_index[1], dtype=np.int64)
    E = row.shape[0]

    core = row // RPC
    lrow = row - core * RPC
    t = lrow >> 7
    m = lrow & 127
    ch = col // CHSZ
    lcol = col - ch * CHSZ

    gid = (core * NCH + ch) * NT + t
    order = np.argsort(gid, kind="stable")
    gid_s = gid[order]
    lcol_s = lcol[order]
    m_s = m[order]

    ngroups = NCORE * NCH * NT
    cnt = np.bincount(gid, minlength=ngroups).reshape(NCORE, NCH, NT)
    nbt = np.maximum(1, -(-cnt.max(axis=0) // P))      # [NCH, NT]
    seg_len = nbt * P
    Lc = seg_len.sum(axis=1)                            # [NCH]
    seg_start = np.zeros((NCH, NT), np.int64)
    seg_start[:, 1:] = np.cumsum(seg_len, axis=1)[:, :-1]

    starts_flat = np.zeros(ngroups, np.int64)
    starts_flat[1:] = np.cumsum(cnt.reshape(-1))[:-1]
    rank = np.arange(E, dtype=np.int64) - starts_flat[gid_s]

    core_s = gid_s // (NCH * NT)
    ch_s = (gid_s // NT) % NCH
    t_s = gid_s % NT
    dest = seg_start[ch_s, t_s] + rank

    idx_arrs = [[None] * NCH for _ in range(NCORE)]
    m_arrs = [[None] * NCH for _ in range(NCORE)]
    for c in range(NCORE):
        cm = core_s == c
        for k in range(NCH):
            mask = cm & (ch_s == k)
            ia = np.zeros(Lc[k], np.int16)
            ma = np.full(Lc[k], 200.0, np.float32)
            d = dest[mask]
            ia[d] = lcol_s[mask].astype(np.int16)
            ma[d] = m_s[mask].astype(np.float32)
            idx_arrs[c][k] = ia
            m_arrs[c][k] = ma
    return nbt, Lc, idx_arrs, m_arrs


def _build(nbt, Lc):
    nc = bacc.Bacc("TRN2", target_bir_lowering=False, num_swdge_queues=4)
    wt = nc.dram_tensor("wt", [N, D], mybir.dt.bfloat16, kind="ExternalInput")
    bias = nc.dram_tensor("bias", [P, D], mybir.dt.float32, kind="ExternalInput")
    ltot = int(Lc.sum())
    idx_d = nc.dram_tensor("idx", [P, ltot // 16], mybir.dt.int16,
                           kind="ExternalInput")
    m_d = nc.dram_tensor("m", [P, ltot // P], mybir.dt.bfloat16,
                         kind="ExternalInput")
    out = nc.dram_tensor("out", [RPC, D], mybir.dt.float32, kind="ExternalOutput")
    off16 = np.concatenate([[0], np.cumsum(Lc // 16)])
    off128 = np.concatenate([[0], np.cumsum(Lc // P)])

    with tile.TileContext(nc) as tc:
        with tc.tile_pool(name="const", bufs=1) as cpool, \
             tc.tile_pool(name="g", bufs=16) as gpool, \
             tc.tile_pool(name="s", bufs=16) as spool, \
             tc.tile_pool(name="o", bufs=6) as opool, \
             tc.tile_pool(name="psum", bufs=7, space="PSUM") as pspool:

            iota16 = cpool.tile([P, P], mybir.dt.int16)
            nc.gpsimd.iota(iota16[:], pattern=[[1, P]], base=0, channel_multiplier=0)
            iota_bf = cpool.tile([P, P], mybir.dt.bfloat16)
            nc.vector.tensor_copy(iota_bf[:], iota16[:])
            bias_t = cpool.tile([P, D], mybir.dt.float32)
            nc.sync.dma_start(bias_t[:], bias[:])
            idx_all = cpool.tile([P, ltot // 16], mybir.dt.int16)
            nc.sync.dma_start(idx_all[:], idx_d[:])
            m_all = cpool.tile([P, ltot // P], mybir.dt.bfloat16)
            nc.sync.dma_start(m_all[:], m_d[:])


            gq = [0]
            g_tiles = {}
            s_tiles = {}

            def ensure(k, s):
                if (k, s) in g_tiles:
                    return
                lck = int(Lc[k])
                n = min(CALL, lck - s * CALL)
                nb_s = n // P
                g = gpool.tile([P, NBC, D], mybir.dt.bfloat16, tag="g")
                i0 = int(off16[k]) + s * (CALL // 16)
                nc.gpsimd.dma_gather(
                    g[:, :nb_s, :],
                    wt[k * CHSZ:(k + 1) * CHSZ, :],
                    idx_all[:, i0: i0 + n // 16],
                    n, n, D, queue_num=gq[0] % 4,
                )
                gq[0] += 1
                m0 = int(off128[k]) + s * NBC
                st = spool.tile([P, NBC * P], mybir.dt.bfloat16, tag="s")
                nc.vector.tensor_tensor(
                    out=st[:, :nb_s * P].rearrange("p (b m) -> p b m", m=P),
                    in0=m_all[:, m0: m0 + nb_s].unsqueeze(2)
                        .broadcast_to([P, nb_s, P]),
                    in1=iota_bf[:].unsqueeze(1).broadcast_to([P, nb_s, P]),
                    op=mybir.AluOpType.is_equal,
                )
                g_tiles[(k, s)] = g
                s_tiles[(k, s)] = st

            # walk tiles; consume each chunk's stream in order
            bpos = [0] * NCH
            for t in range(NT):
                nb_tot = int(sum(nbt[k][t] for k in range(NCH)))
                ps = pspool.tile([P, D], mybir.dt.float32, space="PSUM")
                j = 0
                for k in range(NCH):
                    for _ in range(int(nbt[k][t])):
                        b = bpos[k]
                        s, b_loc = b // NBC, b % NBC
                        ensure(k, s)
                        nc.tensor.matmul(
                            out=ps[:],
                            lhsT=s_tiles[(k, s)][:, b_loc * P:(b_loc + 1) * P],
                            rhs=g_tiles[(k, s)][:, b_loc, :],
                            start=(j == 0),
                            stop=(j == nb_tot - 1),
                        )
                        bpos[k] += 1
                        j += 1
                rows = P if t < NT - 1 else LAST_TILE_ROWS
                ot = opool.tile([P, D], mybir.dt.float32, tag="o")
                nc.vector.tensor_tensor(
                    out=ot[:], in0=ps[:], in1=bias_t[:],
                    op=mybir.AluOpType.add,
                )
                nc.sync.dma_start(out[t * P:t * P + rows, :], ot[:rows, :])
    nc.compile()
    return nc


def kernel(x=None, edge_index=None, W_weight=None, W_bias=None, _trace=False):
    global LAST_EXEC_NS
    edge_index = np.asarray(edge_index)
    W_weight = np.asarray(W_weight, dtype=np.float32)
    W_bias = np.asarray(W_bias, dtype=np.float32)

    key = (edge_index.tobytes()[:4096], edge_index.shape)
    cached = _CACHE.get(key)
    if cached is None:
        nbt, Lc, idx_arrs, m_arrs = _prepare(edge_index)
        nc = _build(nbt, Lc)
        wt_bf = np.ascontiguousarray(W_weight.T).astype(ml_dtypes.bfloat16)
        bias_b = np.tile(W_bias[None, :], (P, 1)).astype(np.float32)
        in_maps = []
        for c in range(NCORE):
            idx_cat = np.concatenate(
                [np.tile(idx_arrs[c][k].reshape(-1, 16).T, (8, 1))
                 for k in range(NCH)], axis=1)
            m_cat = np.concatenate(
                [m_arrs[c][k].reshape(-1, P).T for k in range(NCH)], axis=1)
            im = {
                "wt": wt_bf,
                "bias": bias_b,
                "idx": np.ascontiguousarray(idx_cat).astype(np.int16),
                "m": np.ascontiguousarray(m_cat).astype(ml_dtypes.bfloat16),
            }
            in_maps.append(im)
        _CACHE[key] = (nc, in_maps)
    else:
        nc, in_maps = cached

    res = run_bass_kernel_spmd(nc, in_maps, core_ids=list(range(NCORE)),
                               trace=_trace)
    LAST_EXEC_NS = res.exec_time_ns
    outp = np.concatenate([res.results[c]["out"] for c in range(NCORE)], axis=0)
    return outp.astype(np.float32)


# revision 3
# speedup vs baseline: 1.1025x; 1.0060x over previous
"""LINK forward (gnn message passing SpMM) on 8 TRN2 NeuronCores — v4.

out[r, :] = W_bias + sum_{e: row[e]==r} W_weight.T[col[e], :]

v4 over v3:
  - Global 128-row tiles (782) are load-balanced across the 8 cores by
    sorted block count, so the SPMD shared schedule has near-zero
    max-over-cores padding. Host reassembles rows.
  - Edges within a tile are sorted by col in alternating (zigzag)
    direction per tile slot, so every 1024-idx gather call covers a
    <32768-wide col window with a per-call wt base: int16 indices with
    no 4-chunk ceil padding and all calls full-size.
  - Per-tile PSUM accumulation, bias fused in the PSUM->SBUF copy,
    output DMA'd per tile.
"""

import sys

sys.path.insert(0, "/opt/trn_rl_repo")

import numpy as np
import ml_dtypes

import concourse.bass as bass
import concourse.tile as tile
from concourse import bacc, mybir
from concourse.bass_utils import run_bass_kernel_spmd

P = 128
D = 128
N = 100000
NCORE = 8
NTG = (N + P - 1) // P         # 782 global tiles (last has 32 rows)
SLOTS = (NTG + NCORE - 1) // NCORE  # 98 tile slots per core
CALL = 1024
NBC = CALL // P                # 8 blocks per full call
WIN = 32768                    # int16 col window

LAST_EXEC_NS = None
_CACHE = {}


def _prepare(edge_index):
    """Returns (nbs [SLOTS] common block counts, tile_map [NCORE][SLOTS],
    calls [(idx_off, n_idx, base, src_rows, blk_off)], idx_arr [NCORE, ltot]
    int16, m_arr [NCORE, ltot] f32)."""
    row = np.asarray(edge_index[0], dtype=np.int64)
    col = np.asarray(edge_index[1], dtype=np.int64)

    tg = row >> 7
    m = row & 127
    cnt = np.bincount(tg, minlength=NTG)                 # [782]
    nb = np.maximum(1, -(-cnt // P))                     # blocks per tile

    # snake-assign tiles (sorted by nb desc) to NCORE x SLOTS grid
    order = np.argsort(-nb, kind="stable")
    tile_map = np.full((NCORE, SLOTS), -1, np.int64)
    for j in range(SLOTS):
        chunk = order[j * NCORE:(j + 1) * NCORE]
        cores = range(NCORE) if j % 2 == 0 else range(NCORE - 1, -1, -1)
        for ci, c in enumerate(cores):
            if ci < len(chunk):
                tile_map[c, j] = chunk[ci]
    nbs = np.zeros(SLOTS, np.int64)
    for j in range(SLOTS):
        ts = tile_map[:, j]
        nbs[j] = max(int(nb[t]) if t >= 0 else 1 for t in ts)
    blk_start = np.zeros(SLOTS, np.int64)
    blk_start[1:] = np.cumsum(nbs)[:-1]
    nblk_tot = int(nbs.sum())
    ltot = nblk_tot * P

    # per-core streams: cols (zigzag-sorted) + m, padded per slot
    colstr = np.zeros((NCORE, ltot), np.int64)   # absolute col per slot
    validm = np.zeros((NCORE, ltot), bool)
    m_arr = np.full((NCORE, ltot), 200.0, np.float32)

    order_e = np.argsort(tg, kind="stable")
    tg_s = tg[order_e]
    col_s = col[order_e]
    m_s = m[order_e]
    tstart = np.zeros(NTG + 1, np.int64)
    tstart[1:] = np.cumsum(cnt)

    for c in range(NCORE):
        for j in range(SLOTS):
            t = int(tile_map[c, j])
            off = int(blk_start[j]) * P
            if t < 0:
                continue
            a, b = int(tstart[t]), int(tstart[t + 1])
            cc = col_s[a:b]
            mm = m_s[a:b]
            o = np.argsort(cc, kind="stable")
            if j % 2 == 1:
                o = o[::-1]
            n = b - a
            if n == 0:
                continue
            # stretch the sorted stream across the slot's common length so
            # every core's col progression aligns per block
            L = int(nbs[j]) * P
            pos = off + np.arange(n, dtype=np.int64) * L // n
            colstr[c, pos] = cc[o]
            m_arr[c, pos] = mm[o]
            validm[c, pos] = True

    # call splitting: walk blocks, cut at CALL blocks or window overflow
    calls = []
    idx_arr = np.zeros((NCORE, ltot), np.int16)
    b0 = 0
    while b0 < nblk_tot:
        be = min(b0 + NBC, nblk_tot)
        while True:
            lo, hi = N, 0
            s0, s1 = b0 * P, be * P
            for c in range(NCORE):
                v = validm[c, s0:s1]
                if v.any():
                    cs = colstr[c, s0:s1][v]
                    lo = min(lo, int(cs.min()))
                    hi = max(hi, int(cs.max()))
            if hi < lo:
                lo = 0
                break
            if hi - lo < WIN:
                break
            be -= 1
            assert be > b0, "single block exceeds col window"
        base = lo
        src_rows = min(WIN, N - base)
        n_idx = (be - b0) * P
        for c in range(NCORE):
            s0, s1 = b0 * P, be * P
            rel = colstr[c, s0:s1] - base
            rel[~validm[c, s0:s1]] = 0
            assert rel.min() >= 0 and rel.max() < src_rows
            idx_arr[c, s0:s1] = rel.astype(np.int16)
        calls.append((b0 * P, n_idx, base, src_rows, b0))
        b0 = be

    blk2call = np.zeros(nblk_tot, np.int64)
    for ci, (_, n_idx, _, _, blk_off) in enumerate(calls):
        blk2call[blk_off:blk_off + n_idx // P] = ci

    return nbs, tile_map, calls, blk2call, idx_arr, m_arr


def _build(nbs, calls, blk2call):
    ltot = int(nbs.sum()) * P
    nc = bacc.Bacc("TRN2", target_bir_lowering=False, num_swdge_queues=4)
    wt = nc.dram_tensor("wt", [N, D], mybir.dt.bfloat16, kind="ExternalInput")
    bias = nc.dram_tensor("bias", [P, D], mybir.dt.float32, kind="ExternalInput")
    idx_d = nc.dram_tensor("idx", [P, ltot // 16], mybir.dt.int16,
                           kind="ExternalInput")
    m_d = nc.dram_tensor("m", [P, ltot // P], mybir.dt.bfloat16,
                         kind="ExternalInput")
    out = nc.dram_tensor("out", [SLOTS * P, D], mybir.dt.float32,
                         kind="ExternalOutput")

    with tile.TileContext(nc) as tc:
        with tc.tile_pool(name="const", bufs=1) as cpool, \
             tc.tile_pool(name="g", bufs=16) as gpool, \
             tc.tile_pool(name="s", bufs=16) as spool, \
             tc.tile_pool(name="o", bufs=6) as opool, \
             tc.tile_pool(name="psum", bufs=7, space="PSUM") as pspool:

            iota16 = cpool.tile([P, P], mybir.dt.int16)
            nc.gpsimd.iota(iota16[:], pattern=[[1, P]], base=0, channel_multiplier=0)
            iota_bf = cpool.tile([P, P], mybir.dt.bfloat16)
            nc.vector.tensor_copy(iota_bf[:], iota16[:])
            bias_t = cpool.tile([P, D], mybir.dt.float32)
            nc.sync.dma_start(bias_t[:], bias[:])
            idx_all = cpool.tile([P, ltot // 16], mybir.dt.int16)
            nc.sync.dma_start(idx_all[:], idx_d[:])
            m_all = cpool.tile([P, ltot // P], mybir.dt.bfloat16)
            nc.sync.dma_start(m_all[:], m_d[:])

            gq = [0]
            g_tiles = {}
            s_tiles = {}

            def ensure(ci):
                if ci in g_tiles:
                    return
                idx_off, n_idx, base, src_rows, blk_off = calls[ci]
                nb_s = n_idx // P
                g = gpool.tile([P, NBC, D], mybir.dt.bfloat16, tag="g")
                nc.gpsimd.dma_gather(
                    g[:, :nb_s, :],
                    wt[base:base + src_rows, :],
                    idx_all[:, idx_off // 16: (idx_off + n_idx) // 16],
                    n_idx, n_idx, D, queue_num=gq[0] % 4,
                )
                gq[0] += 1
                st = spool.tile([P, NBC * P], mybir.dt.bfloat16, tag="s")
                nc.vector.tensor_tensor(
                    out=st[:, :n_idx].rearrange("p (b m) -> p b m", m=P),
                    in0=m_all[:, blk_off: blk_off + nb_s].unsqueeze(2)
                        .broadcast_to([P, nb_s, P]),
                    in1=iota_bf[:].unsqueeze(1).broadcast_to([P, nb_s, P]),
                    op=mybir.AluOpType.is_equal,
                )
                g_tiles[ci] = g
                s_tiles[ci] = st

            b = 0
            for j in range(SLOTS):
                nbj = int(nbs[j])
                ps = pspool.tile([P, D], mybir.dt.float32, space="PSUM")
                for i in range(nbj):
                    ci = int(blk2call[b])
                    b_loc = b - calls[ci][4]
                    ensure(ci)
                    nc.tensor.matmul(
                        out=ps[:],
                        lhsT=s_tiles[ci][:, b_loc * P:(b_loc + 1) * P],
                        rhs=g_tiles[ci][:, b_loc, :],
                        start=(i == 0),
                        stop=(i == nbj - 1),
                    )
                    b += 1
                ot = opool.tile([P, D], mybir.dt.float32, tag="o")
                nc.vector.tensor_tensor(
                    out=ot[:], in0=ps[:], in1=bias_t[:],
                    op=mybir.AluOpType.add,
                )
                nc.sync.dma_start(out[j * P:(j + 1) * P, :], ot[:])
    nc.compile()
    return nc


def kernel(x=None, edge_index=None, W_weight=None, W_bias=None, _trace=False):
    global LAST_EXEC_NS
    edge_index = np.asarray(edge_index)
    W_weight = np.asarray(W_weight, dtype=np.float32)
    W_bias = np.asarray(W_bias, dtype=np.float32)

    key = (edge_index.tobytes()[:4096], edge_index.shape)
    cached = _CACHE.get(key)
    if cached is None:
        nbs, tile_map, calls, blk2call, idx_arr, m_arr = _prepare(edge_index)
        nc = _build(nbs, calls, blk2call)
        wt_bf = np.ascontiguousarray(W_weight.T).astype(ml_dtypes.bfloat16)
        bias_b = np.tile(W_bias[None, :], (P, 1)).astype(np.float32)
        in_maps = []
        for c in range(NCORE):
            im = {
                "wt": wt_bf,
                "bias": bias_b,
                "idx": np.ascontiguousarray(
                    np.tile(idx_arr[c].reshape(-1, 16).T, (8, 1))
                ).astype(np.int16),
                "m": np.ascontiguousarray(
                    m_arr[c].reshape(-1, P).T).astype(ml_dtypes.bfloat16),
            }
            in_maps.append(im)
        _CACHE[key] = (nc, in_maps, tile_map)
    else:
        nc, in_maps, tile_map = cached

    res = run_bass_kernel_spmd(nc, in_maps, core_ids=list(range(NCORE)),
                               trace=_trace)
    LAST_EXEC_NS = res.exec_time_ns
    outp = np.zeros((N, D), np.float32)
    for c in range(NCORE):
        r = res.results[c]["out"].astype(np.float32)
        for j in range(SLOTS):
            t = int(tile_map[c, j])
            if t < 0:
                continue
            rows = min(P, N - t * P)
            outp[t * P: t * P + rows] = r[j * P: j * P + rows]
    return outp
